# revision 27
# baseline (speedup 1.0000x reference)
"""Multi-head GAT layer on 8 Trainium2 NeuronCores (Bass/Tile).

Strategy: nodes sharded 6250/core; each core owns all edges whose dst is in
its shard. Edges are host-binned into 49 windows of 128 dst slots. The host
stages edge-ordered copies of h (the halo exchange) so the device does only
contiguous DMA loads; per window the device computes el/er attention logits
with per-column matmuls accumulating in PSUM, exponentiates, builds weighted
one-hot matrices, and scatter-accumulates sum_e w_e*h[src_e] per dst slot via
matmuls oriented so the result lands feature-major (no transposes). The output
projection uses the folded weight W_c^T @ wsc_c; biases fold into constants.
"""
import sys, os, types, ctypes, contextlib

if '/opt/trn_rl_repo' not in sys.path:
    sys.path.insert(0, '/opt/trn_rl_repo')


def _install_profile_hook():
    try:
        import antenv.axon_hooks  # noqa
        return
    except ImportError:
        pass
    try:
        import antenv
    except ImportError:
        return
    so_path = "/opt/axon/libaxon_pjrt.so"
    hook = None
    if os.path.exists(so_path):
        lib = ctypes.CDLL(so_path)
        if hasattr(lib, "axon_start_nrt_profile"):
            lib.axon_start_nrt_profile.argtypes = [ctypes.POINTER(ctypes.c_int64), ctypes.c_size_t]
            lib.axon_start_nrt_profile.restype = ctypes.c_int64
            lib.axon_stop_nrt_profile.argtypes = [ctypes.c_char_p]
            lib.axon_stop_nrt_profile.restype = ctypes.c_int64

            @contextlib.contextmanager
            def _hook(output_dir, device_ids):
                import jax
                jax.devices()
                if device_ids:
                    ids = (ctypes.c_int64 * len(device_ids))(*device_ids)
                    rc = lib.axon_start_nrt_profile(ids, len(device_ids))
                else:
                    rc = lib.axon_start_nrt_profile(None, 0)
                if rc != 0:
                    raise RuntimeError(f"axon_start_nrt_profile rc={rc}")
                try:
                    yield
                finally:
                    n = lib.axon_stop_nrt_profile(str(output_dir).encode())
                    print(f"ntff profile: {n} file(s) -> {output_dir}", file=sys.stderr)
            hook = _hook
    mod = types.ModuleType("antenv.axon_hooks")
    state = {"hook": hook}
    mod.set_axon_ntff_profile_hook = lambda h: state.__setitem__("hook", h)
    mod.get_axon_ntff_profile_hook = lambda: state["hook"]
    sys.modules["antenv.axon_hooks"] = mod
    antenv.axon_hooks = mod


_install_profile_hook()

import numpy as np
import ml_dtypes
from concourse import bass, bacc, mybir, tile
from concourse.bass_utils import run_bass_kernel_spmd

N_NODES = 50000
F = 128
H = 3
NCORES = 8
NPC = N_NODES // NCORES          # 6250 nodes per core
NWIN = (NPC + 127) // 128        # 49 windows per core
NEG_SLOPE = 0.2
LN_EPS = 1e-5

f32 = mybir.dt.float32
bf16 = mybir.dt.bfloat16

_PROGRAM_CACHE = {}


def _build_program(T):
    nc = bacc.Bacc("TRN2", target_bir_lowering=False, debug=False,
                   enable_asserts=False, num_devices=NCORES,
                   dynamic_dma_scratch_size=16384)

    he_in = nc.dram_tensor("he", [NWIN, 128, T * F], bf16, kind="ExternalInput").ap()
    heT_in = nc.dram_tensor("heT", [NWIN, 128, T * F], bf16, kind="ExternalInput").ap()
    hdT_in = nc.dram_tensor("hdT", [NWIN, 128, T * F], bf16, kind="ExternalInput").ap()
    hwin_in = nc.dram_tensor("hwin", [NWIN, 128, F], f32, kind="ExternalInput").ap()
    wrel_in = nc.dram_tensor("wrel", [NWIN, 128, T], f32, kind="ExternalInput").ap()
    welr_in = nc.dram_tensor("welr", [F, 6], bf16, kind="ExternalInput").ap()
    wfold_in = nc.dram_tensor("wfold", [F, H * F], bf16, kind="ExternalInput").ap()
    iotar3_in = nc.dram_tensor("iotar3", [128, H * 128], bf16, kind="ExternalInput").ap()
    ccr_in = nc.dram_tensor("ccr", [1, T * H], bf16, kind="ExternalInput").ap()
    ones1_in = nc.dram_tensor("ones1", [1, 128], bf16, kind="ExternalInput").ap()
    gam_in = nc.dram_tensor("gam", [128, F], f32, kind="ExternalInput").ap()
    bet_in = nc.dram_tensor("bet", [128, F], f32, kind="ExternalInput").ap()
    bconst_in = nc.dram_tensor("bconst", [128, F], f32, kind="ExternalInput").ap()

    outy = nc.dram_tensor("outy", [NWIN * 128, F], f32, kind="ExternalOutput").ap()

    with tile.TileContext(nc) as tc:
        with (
            tc.tile_pool(name="const", bufs=1) as cpool,
            tc.tile_pool(name="gath", bufs=3) as gpool,
            tc.tile_pool(name="edge", bufs=4) as epool,
            tc.tile_pool(name="small", bufs=3) as spool,
            tc.tile_pool(name="fin", bufs=3) as fpool,
            tc.tile_pool(name="psat", bufs=2, space="PSUM") as psat,
            tc.tile_pool(name="psgt", bufs=2, space="PSUM") as psgt,
            tc.tile_pool(name="pspp", bufs=2, space="PSUM") as pspp,
        ):
            # ---- constants
            welr = cpool.tile([F, 6], bf16)
            nc.sync.dma_start(welr[:], welr_in[:])
            wfold = cpool.tile([F, H * F], bf16)
            nc.sync.dma_start(wfold[:], wfold_in[:])
            iotar3 = cpool.tile([128, H * 128], bf16)
            nc.sync.dma_start(iotar3[:], iotar3_in[:])
            ccr = cpool.tile([1, T * H], bf16)
            nc.sync.dma_start(ccr[:], ccr_in[:])
            ones1 = cpool.tile([1, 128], bf16)
            nc.sync.dma_start(ones1[:], ones1_in[:])
            gam = cpool.tile([128, F], f32)
            nc.sync.dma_start(gam[:], gam_in[:])
            bet = cpool.tile([128, F], f32)
            nc.sync.dma_start(bet[:], bet_in[:])
            bconst = cpool.tile([128, F], f32)
            nc.sync.dma_start(bconst[:], bconst_in[:])
            onescol = cpool.tile([128, 1], bf16)
            nc.vector.memset(onescol[:], 1.0)
            pt02 = cpool.tile([128, 64], f32)
            nc.vector.memset(pt02[:], NEG_SLOPE)

            for w in range(NWIN):
                ghe = gpool.tile([128, T, F], bf16, tag="ghe")
                nc.sync.dma_start(ghe[:].rearrange("p t f -> p (t f)"), he_in[w])
                gheT = gpool.tile([128, T, F], bf16, tag="gheT")
                nc.scalar.dma_start(gheT[:].rearrange("p t f -> p (t f)"), heT_in[w])
                gdT = gpool.tile([128, T, F], bf16, tag="gdT")
                nc.sync.dma_start(gdT[:].rearrange("p t f -> p (t f)"), hdT_in[w])
                hw = spool.tile([128, F], f32, tag="hw")
                nc.scalar.dma_start(hw[:], hwin_in[w])
                wr = spool.tile([128, T], f32, tag="wr")
                nc.sync.dma_start(wr[:], wrel_in[w])

                # attention logits el[src]+er[dst]+bias accumulate in PSUM
                atp = psat.tile([128, T * H], f32, tag="at")
                nc.tensor.matmul(out=atp[:], lhsT=ones1[:], rhs=ccr[:],
                                 start=True, stop=False)
                for j in range(T):
                    nc.tensor.matmul(out=atp[:, j * H:(j + 1) * H],
                                     lhsT=gheT[:, j, :], rhs=welr[:, 0:3],
                                     start=False, stop=False)
                    nc.tensor.matmul(out=atp[:, j * H:(j + 1) * H],
                                     lhsT=gdT[:, j, :], rhs=welr[:, 3:6],
                                     start=False, stop=(j == T - 1))

                # ew = exp(leaky(attn))
                atsb = spool.tile([128, T * H], f32, tag="atsb")
                nc.scalar.copy(atsb[:], atp[:])
                at2 = spool.tile([128, T * H], f32, tag="at2")
                nc.gpsimd.tensor_tensor(out=at2[:], in0=atsb[:], in1=pt02[:, 0:T * H],
                                        op=mybir.AluOpType.mult)
                at3 = spool.tile([128, T * H], f32, tag="at3")
                nc.vector.tensor_tensor(out=at3[:], in0=atsb[:], in1=at2[:],
                                        op=mybir.AluOpType.max)
                ew = spool.tile([128, T, H], f32, tag="ew")
                nc.scalar.activation(ew[:].rearrange("p t c -> p (t c)"), at3[:],
                                     mybir.ActivationFunctionType.Exp)

                # scatter: GT[f, c*128+slot] += sum_e ghe[e,f] * OW_c[e,slot]
                # den_c[slot] accumulates in cols 384:387 of the same PSUM bank
                gt = psgt.tile([128, H * 128 + H], f32, tag="gt")
                for j in range(T):
                    ow = epool.tile([128, H * 128], bf16, tag="ow")
                    for c in range(H):
                        nc.vector.tensor_scalar(
                            out=ow[:, c * 128:(c + 1) * 128],
                            in0=iotar3[:, c * 128:(c + 1) * 128],
                            scalar1=wr[:, j:j + 1], scalar2=ew[:, j, c:c + 1],
                            op0=mybir.AluOpType.is_equal, op1=mybir.AluOpType.mult)
                    nc.tensor.matmul(out=gt[:, 0:H * 128], lhsT=ghe[:, j, :], rhs=ow[:],
                                     start=(j == 0), stop=False)
                    for c in range(H):
                        nc.tensor.matmul(out=gt[:, H * 128 + c:H * 128 + c + 1],
                                         lhsT=ow[:, c * 128:(c + 1) * 128],
                                         rhs=onescol[:], start=False,
                                         stop=(j == T - 1 and c == H - 1))

                # epilogue: x = sum_c (GT_c^T @ wfold_c) / den_c + hwin + bconst
                dmx = spool.tile([128, H], f32, tag="dmx")
                nc.vector.tensor_scalar(out=dmx[:], in0=gt[:, H * 128:H * 128 + H],
                                        scalar1=1e-9, scalar2=None,
                                        op0=mybir.AluOpType.max)
                dr = spool.tile([128, H], f32, tag="dr")
                nc.vector.reciprocal(dr[:], dmx[:])
                gtsb = epool.tile([128, H * 128], bf16, tag="gtsb")
                nc.scalar.copy(gtsb[:], gt[:, 0:H * 128])
                pp = pspp.tile([128, H * F], f32, tag="pp")
                for c in range(H):
                    nc.tensor.matmul(out=pp[:, c * F:(c + 1) * F],
                                     lhsT=gtsb[:, c * 128:(c + 1) * 128],
                                     rhs=wfold[:, c * F:(c + 1) * F],
                                     start=(c == 0), stop=(c == H - 1))
                xa = fpool.tile([128, F], f32, tag="xa")
                nc.scalar.activation(xa[:], pp[:, 0:F],
                                     mybir.ActivationFunctionType.Copy,
                                     scale=dr[:, 0:1])
                xb = fpool.tile([128, F], f32, tag="xb")
                nc.scalar.activation(xb[:], pp[:, F:2 * F],
                                     mybir.ActivationFunctionType.Copy,
                                     scale=dr[:, 1:2])
                xc_ = fpool.tile([128, F], f32, tag="xc_")
                nc.vector.tensor_scalar_mul(xc_[:], pp[:, 2 * F:3 * F], dr[:, 2:3])
                s1 = fpool.tile([128, F], f32, tag="s1")
                nc.gpsimd.tensor_tensor(out=s1[:], in0=xa[:], in1=xb[:], op=mybir.AluOpType.add)
                s2 = fpool.tile([128, F], f32, tag="s2")
                nc.gpsimd.tensor_tensor(out=s2[:], in0=s1[:], in1=xc_[:], op=mybir.AluOpType.add)
                s3 = fpool.tile([128, F], f32, tag="s3")
                nc.gpsimd.tensor_tensor(out=s3[:], in0=s2[:], in1=hw[:], op=mybir.AluOpType.add)
                x = fpool.tile([128, F], f32, tag="x")
                nc.gpsimd.tensor_tensor(out=x[:], in0=s3[:], in1=bconst[:], op=mybir.AluOpType.add)

                # LayerNorm + relu
                jnk = fpool.tile([128, F], f32, tag="jnk")
                sm = fpool.tile([128, 1], f32, tag="sm")
                nc.scalar.activation(jnk[:], x[:], mybir.ActivationFunctionType.Identity,
                                     accum_out=sm[:, 0:1])
                nmu = fpool.tile([128, 1], f32, tag="nmu")
                nc.vector.tensor_scalar_mul(nmu[:], sm[:], -1.0 / F)
                xm = fpool.tile([128, F], f32, tag="xm")
                nc.scalar.activation(xm[:], x[:], mybir.ActivationFunctionType.Identity,
                                     bias=nmu[:, 0:1], scale=1.0)
                sq = fpool.tile([128, F], f32, tag="sq")
                vs = fpool.tile([128, 1], f32, tag="vs")
                nc.scalar.activation(sq[:], xm[:], mybir.ActivationFunctionType.Square,
                                     accum_out=vs[:, 0:1])
                vp = fpool.tile([128, 1], f32, tag="vp")
                nc.vector.tensor_scalar(out=vp[:], in0=vs[:], scalar1=1.0 / F,
                                        scalar2=LN_EPS, op0=mybir.AluOpType.mult,
                                        op1=mybir.AluOpType.add)
                lvp = fpool.tile([128, 1], f32, tag="lvp")
                nc.scalar.activation(lvp[:], vp[:], mybir.ActivationFunctionType.Ln)
                si = fpool.tile([128, 1], f32, tag="si")
                nc.scalar.activation(si[:], lvp[:], mybir.ActivationFunctionType.Exp,
                                     scale=-0.5)
                y1 = fpool.tile([128, F], f32, tag="y1")
                nc.scalar.activation(y1[:], xm[:], mybir.ActivationFunctionType.Copy,
                                     scale=si[:, 0:1])
                y2 = fpool.tile([128, F], f32, tag="y2")
                nc.gpsimd.tensor_tensor(out=y2[:], in0=y1[:], in1=gam[:], op=mybir.AluOpType.mult)
                y3 = fpool.tile([128, F], f32, tag="y3")
                nc.gpsimd.tensor_tensor(out=y3[:], in0=y2[:], in1=bet[:], op=mybir.AluOpType.add)
                y4 = fpool.tile([128, F], f32, tag="y4")
                nc.scalar.activation(y4[:], y3[:], mybir.ActivationFunctionType.Relu)
                nc.sync.dma_start(outy[w * 128:(w + 1) * 128, :], y4[:])

    nc.compile()
    return nc


def _host_prep(h, src, dst, W_node, b_node, att, w_scale, bias, ln_gamma, ln_beta):
    src = np.asarray(src).astype(np.int64)
    dst = np.asarray(dst).astype(np.int64)
    h = np.asarray(h, dtype=np.float32)
    W_node = np.asarray(W_node, dtype=np.float32)
    b_node = np.asarray(b_node, dtype=np.float32)
    att = np.asarray(att, dtype=np.float32)
    w_scale = np.asarray(w_scale, dtype=np.float32)
    bias = np.asarray(bias, dtype=np.float32)
    ln_gamma = np.asarray(ln_gamma, dtype=np.float32)
    ln_beta = np.asarray(ln_beta, dtype=np.float32)

    deg = np.bincount(dst, minlength=N_NODES)

    # per-core window assignment (balance edge load across NWIN windows)
    win_of = np.zeros(N_NODES, np.int32)
    slot_of = np.zeros(N_NODES, np.int32)
    nodeid = np.zeros((NCORES, NWIN, 128), np.int64)
    valid = np.zeros((NCORES, NWIN, 128), bool)
    maxload = 0
    for k in range(NCORES):
        nodes = np.arange(k * NPC, (k + 1) * NPC)
        order = nodes[np.argsort(-deg[nodes], kind="stable")]
        load = np.zeros(NWIN, np.int64)
        cnt = np.zeros(NWIN, np.int64)
        for n in order:
            cand = np.where(cnt < 128)[0]
            b = cand[np.argmin(load[cand])]
            win_of[n] = b
            slot_of[n] = cnt[b]
            nodeid[k, b, cnt[b]] = n
            valid[k, b, cnt[b]] = True
            load[b] += deg[n]
            cnt[b] += 1
        maxload = max(maxload, load.max())
    T = max(1, int(-(-maxload // 128)))

    sidx = np.zeros((NCORES, NWIN, 128, T), np.int64)
    didx = np.zeros((NCORES, NWIN, 128, T), np.int64)
    wrel = np.full((NCORES, NWIN, 128, T), 255.0, np.float32)

    core_of_edge = dst // NPC
    win_of_edge = win_of[dst]
    for k in range(NCORES):
        em = core_of_edge == k
        for w in range(NWIN):
            sel = em & (win_of_edge == w)
            es = src[sel]
            ed = dst[sel]
            ne = es.shape[0]
            cap = T * 128
            assert ne <= cap
            sarr = np.zeros(cap, np.int64)
            darr = np.zeros(cap, np.int64)
            rarr = np.full(cap, 255.0, np.float32)
            sarr[:ne] = es
            darr[:ne] = ed
            rarr[:ne] = slot_of[ed]
            sidx[k, w] = sarr.reshape(T, 128).T
            didx[k, w] = darr.reshape(T, 128).T
            wrel[k, w] = rarr.reshape(T, 128).T

    # weight-derived constants
    Wn3 = W_node.reshape(H, F, F)                 # (c, f_out, g)
    att_l = att[:, :F]
    att_r = att[:, F:]
    Ael = np.einsum('hfg,hf->gh', Wn3, att_l)     # [g, H]
    Aer = np.einsum('hfg,hf->gh', Wn3, att_r)
    welr = np.concatenate([Ael, Aer], axis=1).astype(ml_dtypes.bfloat16)  # [F, 6]
    b3 = b_node.reshape(H, F)
    cel = (b3 * att_l).sum(1)
    cer = (b3 * att_r).sum(1)
    ccr = np.tile((cel + cer)[None, :], (1, T)).astype(ml_dtypes.bfloat16)  # [1, T*H]

    wfold = np.zeros((F, H * F), np.float32)
    for c in range(H):
        Wc = W_node[c * F:(c + 1) * F, :]
        wsc_c = w_scale[c * F:(c + 1) * F, :]
        wfold[:, c * F:(c + 1) * F] = Wc.T @ wsc_c
    bconst_row = b_node @ w_scale + bias

    hbf = h.astype(ml_dtypes.bfloat16)
    iotar3 = np.tile(np.arange(128, dtype=np.float32)[None, :],
                     (128, H)).astype(ml_dtypes.bfloat16)

    common = {
        "welr": np.ascontiguousarray(welr),
        "wfold": np.ascontiguousarray(wfold.astype(ml_dtypes.bfloat16)),
        "iotar3": np.ascontiguousarray(iotar3),
        "ccr": ccr,
        "ones1": np.ones((1, 128), ml_dtypes.bfloat16),
        "gam": np.tile(ln_gamma[None, :], (128, 1)).astype(np.float32),
        "bet": np.tile(ln_beta[None, :], (128, 1)).astype(np.float32),
        "bconst": np.tile(bconst_row[None, :], (128, 1)).astype(np.float32),
    }
    in_maps = []
    for k in range(NCORES):
        he = hbf[sidx[k]]                          # [NWIN, 128, T, F]
        hd = hbf[didx[k]]
        heT = np.ascontiguousarray(he.transpose(0, 3, 2, 1))  # [NWIN, F, T, 128]
        hdT = np.ascontiguousarray(hd.transpose(0, 3, 2, 1))
        hwin = np.zeros((NWIN, 128, F), np.float32)
        hwin[valid[k]] = h[nodeid[k][valid[k]]]
        m = dict(common)
        m["he"] = np.ascontiguousarray(he.reshape(NWIN, 128, T * F))
        m["heT"] = heT.reshape(NWIN, 128, T * F)
        m["hdT"] = hdT.reshape(NWIN, 128, T * F)
        m["hwin"] = hwin
        m["wrel"] = np.ascontiguousarray(wrel[k])
        in_maps.append(m)
    return T, in_maps, nodeid, valid


def kernel(h, src, dst, W_node, b_node, att, w_scale, bias, ln_gamma, ln_beta,
           _want_trace=False):
    T, in_maps, nodeid, valid = _host_prep(
        h, src, dst, W_node, b_node, att, w_scale, bias, ln_gamma, ln_beta)
    if T not in _PROGRAM_CACHE:
        _PROGRAM_CACHE[T] = _build_program(T)
    nc = _PROGRAM_CACHE[T]
    res = run_bass_kernel_spmd(nc, in_maps, list(range(NCORES)), trace=_want_trace)
    out = np.zeros((N_NODES, F), np.float32)
    for k in range(NCORES):
        rows = res.results[k]["outy"].reshape(NWIN, 128, F)
        v = valid[k]
        out[nodeid[k][v]] = rows[v]
    if _want_trace:
        kernel._last_exec_time_ns = res.exec_time_ns
        kernel._last_trace = res.instructions_and_trace
    return out


# revision 28
# speedup vs baseline: 1.0363x; 1.0363x over previous
"""Multi-head GAT layer on 8 Trainium2 NeuronCores (Bass/Tile).

Strategy: nodes sharded 6250/core; each core owns all edges whose dst is in
its shard. Edges are host-binned into 49 windows of 128 dst slots. The host
stages edge-ordered copies of h (the halo exchange) so the device does only
contiguous DMA loads; per window the device computes el/er attention logits
with per-column matmuls accumulating in PSUM, exponentiates, builds weighted
one-hot matrices, and scatter-accumulates sum_e w_e*h[src_e] per dst slot via
matmuls oriented so the result lands feature-major (no transposes). The output
projection uses the folded weight W_c^T @ wsc_c; biases fold into constants.
"""
import sys, os, types, ctypes, contextlib

if '/opt/trn_rl_repo' not in sys.path:
    sys.path.insert(0, '/opt/trn_rl_repo')


def _install_profile_hook():
    try:
        import antenv.axon_hooks  # noqa
        return
    except ImportError:
        pass
    try:
        import antenv
    except ImportError:
        return
    so_path = "/opt/axon/libaxon_pjrt.so"
    hook = None
    if os.path.exists(so_path):
        lib = ctypes.CDLL(so_path)
        if hasattr(lib, "axon_start_nrt_profile"):
            lib.axon_start_nrt_profile.argtypes = [ctypes.POINTER(ctypes.c_int64), ctypes.c_size_t]
            lib.axon_start_nrt_profile.restype = ctypes.c_int64
            lib.axon_stop_nrt_profile.argtypes = [ctypes.c_char_p]
            lib.axon_stop_nrt_profile.restype = ctypes.c_int64

            @contextlib.contextmanager
            def _hook(output_dir, device_ids):
                import jax
                jax.devices()
                if device_ids:
                    ids = (ctypes.c_int64 * len(device_ids))(*device_ids)
                    rc = lib.axon_start_nrt_profile(ids, len(device_ids))
                else:
                    rc = lib.axon_start_nrt_profile(None, 0)
                if rc != 0:
                    raise RuntimeError(f"axon_start_nrt_profile rc={rc}")
                try:
                    yield
                finally:
                    n = lib.axon_stop_nrt_profile(str(output_dir).encode())
                    print(f"ntff profile: {n} file(s) -> {output_dir}", file=sys.stderr)
            hook = _hook
    mod = types.ModuleType("antenv.axon_hooks")
    state = {"hook": hook}
    mod.set_axon_ntff_profile_hook = lambda h: state.__setitem__("hook", h)
    mod.get_axon_ntff_profile_hook = lambda: state["hook"]
    sys.modules["antenv.axon_hooks"] = mod
    antenv.axon_hooks = mod


_install_profile_hook()

import numpy as np
import ml_dtypes
from concourse import bass, bacc, mybir, tile
from concourse.bass_utils import run_bass_kernel_spmd

N_NODES = 50000
F = 128
H = 3
NCORES = 8
NPC = N_NODES // NCORES          # 6250 nodes per core
NWIN = (NPC + 127) // 128        # 49 windows per core
NEG_SLOPE = 0.2
LN_EPS = 1e-5

f32 = mybir.dt.float32
bf16 = mybir.dt.bfloat16

_PROGRAM_CACHE = {}


def _build_program(T):
    nc = bacc.Bacc("TRN2", target_bir_lowering=False, debug=False,
                   enable_asserts=False, num_devices=NCORES,
                   dynamic_dma_scratch_size=16384)

    he_in = nc.dram_tensor("he", [NWIN, 128, T * F], bf16, kind="ExternalInput").ap()
    heT_in = nc.dram_tensor("heT", [NWIN, 128, T * F], bf16, kind="ExternalInput").ap()
    hdT_in = nc.dram_tensor("hdT", [NWIN, 128, T * F], bf16, kind="ExternalInput").ap()
    hwin_in = nc.dram_tensor("hwin", [NWIN, 128, F], f32, kind="ExternalInput").ap()
    wrel_in = nc.dram_tensor("wrel", [NWIN, 128, T], f32, kind="ExternalInput").ap()
    welr_in = nc.dram_tensor("welr", [F, 6], bf16, kind="ExternalInput").ap()
    wfold_in = nc.dram_tensor("wfold", [F, H * F], bf16, kind="ExternalInput").ap()
    iotar3_in = nc.dram_tensor("iotar3", [128, H * 128], bf16, kind="ExternalInput").ap()
    ccr_in = nc.dram_tensor("ccr", [1, T * H], bf16, kind="ExternalInput").ap()
    ones1_in = nc.dram_tensor("ones1", [1, 128], bf16, kind="ExternalInput").ap()
    gam_in = nc.dram_tensor("gam", [128, F], f32, kind="ExternalInput").ap()
    bet_in = nc.dram_tensor("bet", [128, F], f32, kind="ExternalInput").ap()
    bconst_in = nc.dram_tensor("bconst", [128, F], f32, kind="ExternalInput").ap()

    outy = nc.dram_tensor("outy", [NWIN * 128, F], f32, kind="ExternalOutput").ap()

    with tile.TileContext(nc) as tc:
        with (
            tc.tile_pool(name="const", bufs=1) as cpool,
            tc.tile_pool(name="gath", bufs=3) as gpool,
            tc.tile_pool(name="edge", bufs=4) as epool,
            tc.tile_pool(name="small", bufs=3) as spool,
            tc.tile_pool(name="fin", bufs=3) as fpool,
            tc.tile_pool(name="psat", bufs=2, space="PSUM") as psat,
            tc.tile_pool(name="psgt", bufs=2, space="PSUM") as psgt,
            tc.tile_pool(name="pspp", bufs=2, space="PSUM") as pspp,
        ):
            # ---- constants
            welr = cpool.tile([F, 6], bf16)
            nc.sync.dma_start(welr[:], welr_in[:])
            wfold = cpool.tile([F, H * F], bf16)
            nc.sync.dma_start(wfold[:], wfold_in[:])
            iotar3 = cpool.tile([128, H * 128], bf16)
            nc.sync.dma_start(iotar3[:], iotar3_in[:])
            ccr = cpool.tile([1, T * H], bf16)
            nc.sync.dma_start(ccr[:], ccr_in[:])
            ones1 = cpool.tile([1, 128], bf16)
            nc.sync.dma_start(ones1[:], ones1_in[:])
            gam = cpool.tile([128, F], f32)
            nc.sync.dma_start(gam[:], gam_in[:])
            bet = cpool.tile([128, F], f32)
            nc.sync.dma_start(bet[:], bet_in[:])
            bconst = cpool.tile([128, F], f32)
            nc.sync.dma_start(bconst[:], bconst_in[:])
            onescol = cpool.tile([128, 1], bf16)
            nc.vector.memset(onescol[:], 1.0)
            pt02 = cpool.tile([128, 64], f32)
            nc.vector.memset(pt02[:], NEG_SLOPE)

            def stage_attn(w):
                ghe = gpool.tile([128, T, F], bf16, tag="ghe")
                nc.sync.dma_start(ghe[:].rearrange("p t f -> p (t f)"), he_in[w])
                gheT = gpool.tile([128, T, F], bf16, tag="gheT")
                nc.scalar.dma_start(gheT[:].rearrange("p t f -> p (t f)"), heT_in[w])
                gdT = gpool.tile([128, T, F], bf16, tag="gdT")
                nc.sync.dma_start(gdT[:].rearrange("p t f -> p (t f)"), hdT_in[w])
                hw = spool.tile([128, F], f32, tag="hw")
                nc.scalar.dma_start(hw[:], hwin_in[w])
                wr = spool.tile([128, T], f32, tag="wr")
                nc.sync.dma_start(wr[:], wrel_in[w])

                # attention logits el[src]+er[dst]+bias accumulate in PSUM
                atp = psat.tile([128, T * H], f32, tag="at")
                nc.tensor.matmul(out=atp[:], lhsT=ones1[:], rhs=ccr[:],
                                 start=True, stop=False)
                for j in range(T):
                    nc.tensor.matmul(out=atp[:, j * H:(j + 1) * H],
                                     lhsT=gheT[:, j, :], rhs=welr[:, 0:3],
                                     start=False, stop=False)
                    nc.tensor.matmul(out=atp[:, j * H:(j + 1) * H],
                                     lhsT=gdT[:, j, :], rhs=welr[:, 3:6],
                                     start=False, stop=(j == T - 1))

                # ew = exp(leaky(attn))
                atsb = spool.tile([128, T * H], f32, tag="atsb")
                nc.scalar.copy(atsb[:], atp[:])
                at2 = spool.tile([128, T * H], f32, tag="at2")
                nc.gpsimd.tensor_tensor(out=at2[:], in0=atsb[:], in1=pt02[:, 0:T * H],
                                        op=mybir.AluOpType.mult)
                at3 = spool.tile([128, T * H], f32, tag="at3")
                nc.vector.tensor_tensor(out=at3[:], in0=atsb[:], in1=at2[:],
                                        op=mybir.AluOpType.max)
                ew = spool.tile([128, T, H], f32, tag="ew")
                nc.scalar.activation(ew[:].rearrange("p t c -> p (t c)"), at3[:],
                                     mybir.ActivationFunctionType.Exp)
                return ghe, hw, wr, ew

            cur = stage_attn(0)
            for w in range(NWIN):
                ghe, hw, wr, ew = cur
                if w + 1 < NWIN:
                    cur = stage_attn(w + 1)

                # scatter: GT[f, c*128+slot] += sum_e ghe[e,f] * OW_c[e,slot]
                # den_c[slot] accumulates in cols 384:387 of the same PSUM bank
                gt = psgt.tile([128, H * 128 + H], f32, tag="gt")
                for j in range(T):
                    ow = epool.tile([128, H * 128], bf16, tag="ow")
                    for c in range(H):
                        nc.vector.tensor_scalar(
                            out=ow[:, c * 128:(c + 1) * 128],
                            in0=iotar3[:, c * 128:(c + 1) * 128],
                            scalar1=wr[:, j:j + 1], scalar2=ew[:, j, c:c + 1],
                            op0=mybir.AluOpType.is_equal, op1=mybir.AluOpType.mult)
                    nc.tensor.matmul(out=gt[:, 0:H * 128], lhsT=ghe[:, j, :], rhs=ow[:],
                                     start=(j == 0), stop=False)
                    for c in range(H):
                        nc.tensor.matmul(out=gt[:, H * 128 + c:H * 128 + c + 1],
                                         lhsT=ow[:, c * 128:(c + 1) * 128],
                                         rhs=onescol[:], start=False,
                                         stop=(j == T - 1 and c == H - 1))

                # epilogue: x = sum_c (GT_c^T @ wfold_c) / den_c + hwin + bconst
                dmx = spool.tile([128, H], f32, tag="dmx")
                nc.vector.tensor_scalar(out=dmx[:], in0=gt[:, H * 128:H * 128 + H],
                                        scalar1=1e-9, scalar2=None,
                                        op0=mybir.AluOpType.max)
                dr = spool.tile([128, H], f32, tag="dr")
                nc.vector.reciprocal(dr[:], dmx[:])
                gtsb = epool.tile([128, H * 128], bf16, tag="gtsb")
                nc.scalar.copy(gtsb[:], gt[:, 0:H * 128])
                pp = pspp.tile([128, H * F], f32, tag="pp")
                for c in range(H):
                    nc.tensor.matmul(out=pp[:, c * F:(c + 1) * F],
                                     lhsT=gtsb[:, c * 128:(c + 1) * 128],
                                     rhs=wfold[:, c * F:(c + 1) * F],
                                     start=(c == 0), stop=(c == H - 1))
                xa = fpool.tile([128, F], f32, tag="xa")
                nc.scalar.activation(xa[:], pp[:, 0:F],
                                     mybir.ActivationFunctionType.Copy,
                                     scale=dr[:, 0:1])
                xb = fpool.tile([128, F], f32, tag="xb")
                nc.scalar.activation(xb[:], pp[:, F:2 * F],
                                     mybir.ActivationFunctionType.Copy,
                                     scale=dr[:, 1:2])
                xc_ = fpool.tile([128, F], f32, tag="xc_")
                nc.vector.tensor_scalar_mul(xc_[:], pp[:, 2 * F:3 * F], dr[:, 2:3])
                s1 = fpool.tile([128, F], f32, tag="s1")
                nc.gpsimd.tensor_tensor(out=s1[:], in0=xa[:], in1=xb[:], op=mybir.AluOpType.add)
                s2 = fpool.tile([128, F], f32, tag="s2")
                nc.gpsimd.tensor_tensor(out=s2[:], in0=s1[:], in1=xc_[:], op=mybir.AluOpType.add)
                s3 = fpool.tile([128, F], f32, tag="s3")
                nc.gpsimd.tensor_tensor(out=s3[:], in0=s2[:], in1=hw[:], op=mybir.AluOpType.add)
                x = fpool.tile([128, F], f32, tag="x")
                nc.gpsimd.tensor_tensor(out=x[:], in0=s3[:], in1=bconst[:], op=mybir.AluOpType.add)

                # LayerNorm + relu
                jnk = fpool.tile([128, F], f32, tag="jnk")
                sm = fpool.tile([128, 1], f32, tag="sm")
                nc.scalar.activation(jnk[:], x[:], mybir.ActivationFunctionType.Identity,
                                     accum_out=sm[:, 0:1])
                nmu = fpool.tile([128, 1], f32, tag="nmu")
                nc.vector.tensor_scalar_mul(nmu[:], sm[:], -1.0 / F)
                xm = fpool.tile([128, F], f32, tag="xm")
                nc.scalar.activation(xm[:], x[:], mybir.ActivationFunctionType.Identity,
                                     bias=nmu[:, 0:1], scale=1.0)
                sq = fpool.tile([128, F], f32, tag="sq")
                vs = fpool.tile([128, 1], f32, tag="vs")
                nc.scalar.activation(sq[:], xm[:], mybir.ActivationFunctionType.Square,
                                     accum_out=vs[:, 0:1])
                vp = fpool.tile([128, 1], f32, tag="vp")
                nc.vector.tensor_scalar(out=vp[:], in0=vs[:], scalar1=1.0 / F,
                                        scalar2=LN_EPS, op0=mybir.AluOpType.mult,
                                        op1=mybir.AluOpType.add)
                lvp = fpool.tile([128, 1], f32, tag="lvp")
                nc.scalar.activation(lvp[:], vp[:], mybir.ActivationFunctionType.Ln)
                si = fpool.tile([128, 1], f32, tag="si")
                nc.scalar.activation(si[:], lvp[:], mybir.ActivationFunctionType.Exp,
                                     scale=-0.5)
                y1 = fpool.tile([128, F], f32, tag="y1")
                nc.scalar.activation(y1[:], xm[:], mybir.ActivationFunctionType.Copy,
                                     scale=si[:, 0:1])
                y2 = fpool.tile([128, F], f32, tag="y2")
                nc.gpsimd.tensor_tensor(out=y2[:], in0=y1[:], in1=gam[:], op=mybir.AluOpType.mult)
                y3 = fpool.tile([128, F], f32, tag="y3")
                nc.gpsimd.tensor_tensor(out=y3[:], in0=y2[:], in1=bet[:], op=mybir.AluOpType.add)
                y4 = fpool.tile([128, F], f32, tag="y4")
                nc.scalar.activation(y4[:], y3[:], mybir.ActivationFunctionType.Relu)
                nc.sync.dma_start(outy[w * 128:(w + 1) * 128, :], y4[:])

    nc.compile()
    return nc


def _host_prep(h, src, dst, W_node, b_node, att, w_scale, bias, ln_gamma, ln_beta):
    src = np.asarray(src).astype(np.int64)
    dst = np.asarray(dst).astype(np.int64)
    h = np.asarray(h, dtype=np.float32)
    W_node = np.asarray(W_node, dtype=np.float32)
    b_node = np.asarray(b_node, dtype=np.float32)
    att = np.asarray(att, dtype=np.float32)
    w_scale = np.asarray(w_scale, dtype=np.float32)
    bias = np.asarray(bias, dtype=np.float32)
    ln_gamma = np.asarray(ln_gamma, dtype=np.float32)
    ln_beta = np.asarray(ln_beta, dtype=np.float32)

    deg = np.bincount(dst, minlength=N_NODES)

    # per-core window assignment (balance edge load across NWIN windows)
    win_of = np.zeros(N_NODES, np.int32)
    slot_of = np.zeros(N_NODES, np.int32)
    nodeid = np.zeros((NCORES, NWIN, 128), np.int64)
    valid = np.zeros((NCORES, NWIN, 128), bool)
    maxload = 0
    for k in range(NCORES):
        nodes = np.arange(k * NPC, (k + 1) * NPC)
        order = nodes[np.argsort(-deg[nodes], kind="stable")]
        load = np.zeros(NWIN, np.int64)
        cnt = np.zeros(NWIN, np.int64)
        for n in order:
            cand = np.where(cnt < 128)[0]
            b = cand[np.argmin(load[cand])]
            win_of[n] = b
            slot_of[n] = cnt[b]
            nodeid[k, b, cnt[b]] = n
            valid[k, b, cnt[b]] = True
            load[b] += deg[n]
            cnt[b] += 1
        maxload = max(maxload, load.max())
    T = max(1, int(-(-maxload // 128)))

    sidx = np.zeros((NCORES, NWIN, 128, T), np.int64)
    didx = np.zeros((NCORES, NWIN, 128, T), np.int64)
    wrel = np.full((NCORES, NWIN, 128, T), 255.0, np.float32)

    core_of_edge = dst // NPC
    win_of_edge = win_of[dst]
    for k in range(NCORES):
        em = core_of_edge == k
        for w in range(NWIN):
            sel = em & (win_of_edge == w)
            es = src[sel]
            ed = dst[sel]
            ne = es.shape[0]
            cap = T * 128
            assert ne <= cap
            sarr = np.zeros(cap, np.int64)
            darr = np.zeros(cap, np.int64)
            rarr = np.full(cap, 255.0, np.float32)
            sarr[:ne] = es
            darr[:ne] = ed
            rarr[:ne] = slot_of[ed]
            sidx[k, w] = sarr.reshape(T, 128).T
            didx[k, w] = darr.reshape(T, 128).T
            wrel[k, w] = rarr.reshape(T, 128).T

    # weight-derived constants
    Wn3 = W_node.reshape(H, F, F)                 # (c, f_out, g)
    att_l = att[:, :F]
    att_r = att[:, F:]
    Ael = np.einsum('hfg,hf->gh', Wn3, att_l)     # [g, H]
    Aer = np.einsum('hfg,hf->gh', Wn3, att_r)
    welr = np.concatenate([Ael, Aer], axis=1).astype(ml_dtypes.bfloat16)  # [F, 6]
    b3 = b_node.reshape(H, F)
    cel = (b3 * att_l).sum(1)
    cer = (b3 * att_r).sum(1)
    ccr = np.tile((cel + cer)[None, :], (1, T)).astype(ml_dtypes.bfloat16)  # [1, T*H]

    wfold = np.zeros((F, H * F), np.float32)
    for c in range(H):
        Wc = W_node[c * F:(c + 1) * F, :]
        wsc_c = w_scale[c * F:(c + 1) * F, :]
        wfold[:, c * F:(c + 1) * F] = Wc.T @ wsc_c
    bconst_row = b_node @ w_scale + bias

    hbf = h.astype(ml_dtypes.bfloat16)
    iotar3 = np.tile(np.arange(128, dtype=np.float32)[None, :],
                     (128, H)).astype(ml_dtypes.bfloat16)

    common = {
        "welr": np.ascontiguousarray(welr),
        "wfold": np.ascontiguousarray(wfold.astype(ml_dtypes.bfloat16)),
        "iotar3": np.ascontiguousarray(iotar3),
        "ccr": ccr,
        "ones1": np.ones((1, 128), ml_dtypes.bfloat16),
        "gam": np.tile(ln_gamma[None, :], (128, 1)).astype(np.float32),
        "bet": np.tile(ln_beta[None, :], (128, 1)).astype(np.float32),
        "bconst": np.tile(bconst_row[None, :], (128, 1)).astype(np.float32),
    }
    in_maps = []
    for k in range(NCORES):
        he = hbf[sidx[k]]                          # [NWIN, 128, T, F]
        hd = hbf[didx[k]]
        heT = np.ascontiguousarray(he.transpose(0, 3, 2, 1))  # [NWIN, F, T, 128]
        hdT = np.ascontiguousarray(hd.transpose(0, 3, 2, 1))
        hwin = np.zeros((NWIN, 128, F), np.float32)
        hwin[valid[k]] = h[nodeid[k][valid[k]]]
        m = dict(common)
        m["he"] = np.ascontiguousarray(he.reshape(NWIN, 128, T * F))
        m["heT"] = heT.reshape(NWIN, 128, T * F)
        m["hdT"] = hdT.reshape(NWIN, 128, T * F)
        m["hwin"] = hwin
        m["wrel"] = np.ascontiguousarray(wrel[k])
        in_maps.append(m)
    return T, in_maps, nodeid, valid


def kernel(h, src, dst, W_node, b_node, att, w_scale, bias, ln_gamma, ln_beta,
           _want_trace=False):
    T, in_maps, nodeid, valid = _host_prep(
        h, src, dst, W_node, b_node, att, w_scale, bias, ln_gamma, ln_beta)
    if T not in _PROGRAM_CACHE:
        _PROGRAM_CACHE[T] = _build_program(T)
    nc = _PROGRAM_CACHE[T]
    res = run_bass_kernel_spmd(nc, in_maps, list(range(NCORES)), trace=_want_trace)
    out = np.zeros((N_NODES, F), np.float32)
    for k in range(NCORES):
        rows = res.results[k]["outy"].reshape(NWIN, 128, F)
        v = valid[k]
        out[nodeid[k][v]] = rows[v]
    if _want_trace:
        kernel._last_exec_time_ns = res.exec_time_ns
        kernel._last_trace = res.instructions_and_trace
    return out


# revision 29
# speedup vs baseline: 1.0698x; 1.0324x over previous
"""Multi-head GAT layer on 8 Trainium2 NeuronCores (Bass/Tile).

Strategy: nodes sharded 6250/core; each core owns all edges whose dst is in
its shard. Edges are host-binned into 49 windows of 128 dst slots. The host
stages edge-ordered copies of h (the halo exchange) so the device does only
contiguous DMA loads; per window the device computes el/er attention logits
with per-column matmuls accumulating in PSUM, exponentiates, builds weighted
one-hot matrices, and scatter-accumulates sum_e w_e*h[src_e] per dst slot via
matmuls oriented so the result lands feature-major (no transposes). The output
projection uses the folded weight W_c^T @ wsc_c; biases fold into constants.
"""
import sys, os, types, ctypes, contextlib

if '/opt/trn_rl_repo' not in sys.path:
    sys.path.insert(0, '/opt/trn_rl_repo')


def _install_profile_hook():
    try:
        import antenv.axon_hooks  # noqa
        return
    except ImportError:
        pass
    try:
        import antenv
    except ImportError:
        return
    so_path = "/opt/axon/libaxon_pjrt.so"
    hook = None
    if os.path.exists(so_path):
        lib = ctypes.CDLL(so_path)
        if hasattr(lib, "axon_start_nrt_profile"):
            lib.axon_start_nrt_profile.argtypes = [ctypes.POINTER(ctypes.c_int64), ctypes.c_size_t]
            lib.axon_start_nrt_profile.restype = ctypes.c_int64
            lib.axon_stop_nrt_profile.argtypes = [ctypes.c_char_p]
            lib.axon_stop_nrt_profile.restype = ctypes.c_int64

            @contextlib.contextmanager
            def _hook(output_dir, device_ids):
                import jax
                jax.devices()
                if device_ids:
                    ids = (ctypes.c_int64 * len(device_ids))(*device_ids)
                    rc = lib.axon_start_nrt_profile(ids, len(device_ids))
                else:
                    rc = lib.axon_start_nrt_profile(None, 0)
                if rc != 0:
                    raise RuntimeError(f"axon_start_nrt_profile rc={rc}")
                try:
                    yield
                finally:
                    n = lib.axon_stop_nrt_profile(str(output_dir).encode())
                    print(f"ntff profile: {n} file(s) -> {output_dir}", file=sys.stderr)
            hook = _hook
    mod = types.ModuleType("antenv.axon_hooks")
    state = {"hook": hook}
    mod.set_axon_ntff_profile_hook = lambda h: state.__setitem__("hook", h)
    mod.get_axon_ntff_profile_hook = lambda: state["hook"]
    sys.modules["antenv.axon_hooks"] = mod
    antenv.axon_hooks = mod


_install_profile_hook()

import numpy as np
import ml_dtypes
from concourse import bass, bacc, mybir, tile
from concourse.bass_utils import run_bass_kernel_spmd

N_NODES = 50000
F = 128
H = 3
NCORES = 8
NPC = N_NODES // NCORES          # 6250 nodes per core
NWIN = (NPC + 127) // 128        # 49 windows per core
NEG_SLOPE = 0.2
LN_EPS = 1e-5

f32 = mybir.dt.float32
bf16 = mybir.dt.bfloat16

_PROGRAM_CACHE = {}


def _build_program(T):
    nc = bacc.Bacc("TRN2", target_bir_lowering=False, debug=False,
                   enable_asserts=False, num_devices=NCORES,
                   dynamic_dma_scratch_size=16384)

    he_in = nc.dram_tensor("he", [NWIN, 128, T * F], bf16, kind="ExternalInput").ap()
    heT_in = nc.dram_tensor("heT", [NWIN, 128, T * F], bf16, kind="ExternalInput").ap()
    hdT_in = nc.dram_tensor("hdT", [NWIN, 128, T * F], bf16, kind="ExternalInput").ap()
    hwin_in = nc.dram_tensor("hwin", [NWIN, 128, F], f32, kind="ExternalInput").ap()
    wrel_in = nc.dram_tensor("wrel", [NWIN, 128, T], f32, kind="ExternalInput").ap()
    welr_in = nc.dram_tensor("welr", [F, 6], bf16, kind="ExternalInput").ap()
    wfold_in = nc.dram_tensor("wfold", [F, H * F], bf16, kind="ExternalInput").ap()
    iotar3_in = nc.dram_tensor("iotar3", [128, H * 128], bf16, kind="ExternalInput").ap()
    ccr_in = nc.dram_tensor("ccr", [1, T * H], bf16, kind="ExternalInput").ap()
    ones1_in = nc.dram_tensor("ones1", [1, 128], bf16, kind="ExternalInput").ap()
    gam_in = nc.dram_tensor("gam", [128, F], f32, kind="ExternalInput").ap()
    bet_in = nc.dram_tensor("bet", [128, F], f32, kind="ExternalInput").ap()
    bconst_in = nc.dram_tensor("bconst", [128, F], f32, kind="ExternalInput").ap()

    outy = nc.dram_tensor("outy", [NWIN * 128, F], f32, kind="ExternalOutput").ap()

    with tile.TileContext(nc) as tc:
        with (
            tc.tile_pool(name="const", bufs=1) as cpool,
            tc.tile_pool(name="gath", bufs=4) as gpool,
            tc.tile_pool(name="edge", bufs=6) as epool,
            tc.tile_pool(name="small", bufs=4) as spool,
            tc.tile_pool(name="fin", bufs=4) as fpool,
            tc.tile_pool(name="psat", bufs=2, space="PSUM") as psat,
            tc.tile_pool(name="psgt", bufs=3, space="PSUM") as psgt,
            tc.tile_pool(name="pspp", bufs=3, space="PSUM") as pspp,
        ):
            # ---- constants
            welr = cpool.tile([F, 6], bf16)
            nc.sync.dma_start(welr[:], welr_in[:])
            wfold = cpool.tile([F, H * F], bf16)
            nc.sync.dma_start(wfold[:], wfold_in[:])
            iotar3 = cpool.tile([128, H * 128], bf16)
            nc.sync.dma_start(iotar3[:], iotar3_in[:])
            ccr = cpool.tile([1, T * H], bf16)
            nc.sync.dma_start(ccr[:], ccr_in[:])
            ones1 = cpool.tile([1, 128], bf16)
            nc.sync.dma_start(ones1[:], ones1_in[:])
            gam = cpool.tile([128, F], f32)
            nc.sync.dma_start(gam[:], gam_in[:])
            bet = cpool.tile([128, F], f32)
            nc.sync.dma_start(bet[:], bet_in[:])
            bconst = cpool.tile([128, F], f32)
            nc.sync.dma_start(bconst[:], bconst_in[:])
            onescol = cpool.tile([128, 1], bf16)
            nc.vector.memset(onescol[:], 1.0)
            pt02 = cpool.tile([128, 64], f32)
            nc.vector.memset(pt02[:], NEG_SLOPE)

            def stage_attn(w):
                ghe = gpool.tile([128, T, F], bf16, tag="ghe")
                nc.sync.dma_start(ghe[:].rearrange("p t f -> p (t f)"), he_in[w])
                gheT = gpool.tile([128, T, F], bf16, tag="gheT")
                nc.scalar.dma_start(gheT[:].rearrange("p t f -> p (t f)"), heT_in[w])
                gdT = gpool.tile([128, T, F], bf16, tag="gdT")
                nc.sync.dma_start(gdT[:].rearrange("p t f -> p (t f)"), hdT_in[w])
                hw = spool.tile([128, F], f32, tag="hw")
                nc.scalar.dma_start(hw[:], hwin_in[w])
                wr = spool.tile([128, T], f32, tag="wr")
                nc.sync.dma_start(wr[:], wrel_in[w])

                # attention logits el[src]+er[dst]+bias accumulate in PSUM
                atp = psat.tile([128, T * H], f32, tag="at")
                nc.tensor.matmul(out=atp[:], lhsT=ones1[:], rhs=ccr[:],
                                 start=True, stop=False)
                for j in range(T):
                    nc.tensor.matmul(out=atp[:, j * H:(j + 1) * H],
                                     lhsT=gheT[:, j, :], rhs=welr[:, 0:3],
                                     start=False, stop=False)
                    nc.tensor.matmul(out=atp[:, j * H:(j + 1) * H],
                                     lhsT=gdT[:, j, :], rhs=welr[:, 3:6],
                                     start=False, stop=(j == T - 1))

                # ew = exp(leaky(attn))
                atsb = spool.tile([128, T * H], f32, tag="atsb")
                nc.scalar.copy(atsb[:], atp[:])
                at2 = spool.tile([128, T * H], f32, tag="at2")
                nc.gpsimd.tensor_tensor(out=at2[:], in0=atsb[:], in1=pt02[:, 0:T * H],
                                        op=mybir.AluOpType.mult)
                at3 = spool.tile([128, T * H], f32, tag="at3")
                nc.vector.tensor_tensor(out=at3[:], in0=atsb[:], in1=at2[:],
                                        op=mybir.AluOpType.max)
                ew = spool.tile([128, T, H], f32, tag="ew")
                nc.scalar.activation(ew[:].rearrange("p t c -> p (t c)"), at3[:],
                                     mybir.ActivationFunctionType.Exp)
                return ghe, hw, wr, ew

            cur = stage_attn(0)
            for w in range(NWIN):
                ghe, hw, wr, ew = cur
                if w + 1 < NWIN:
                    cur = stage_attn(w + 1)

                # scatter: GT[f, c*128+slot] += sum_e ghe[e,f] * OW_c[e,slot]
                # den_c[slot] accumulates in cols 384:387 of the same PSUM bank
                gt = psgt.tile([128, H * 128 + H], f32, tag="gt")
                for j in range(T):
                    ow = epool.tile([128, H * 128], bf16, tag="ow")
                    for c in range(H):
                        nc.vector.tensor_scalar(
                            out=ow[:, c * 128:(c + 1) * 128],
                            in0=iotar3[:, c * 128:(c + 1) * 128],
                            scalar1=wr[:, j:j + 1], scalar2=ew[:, j, c:c + 1],
                            op0=mybir.AluOpType.is_equal, op1=mybir.AluOpType.mult)
                    nc.tensor.matmul(out=gt[:, 0:H * 128], lhsT=ghe[:, j, :], rhs=ow[:],
                                     start=(j == 0), stop=False)
                    for c in range(H):
                        nc.tensor.matmul(out=gt[:, H * 128 + c:H * 128 + c + 1],
                                         lhsT=ow[:, c * 128:(c + 1) * 128],
                                         rhs=onescol[:], start=False,
                                         stop=(j == T - 1 and c == H - 1))

                # epilogue: x = sum_c (GT_c^T @ wfold_c) / den_c + hwin + bconst
                dmx = spool.tile([128, H], f32, tag="dmx")
                nc.vector.tensor_scalar(out=dmx[:], in0=gt[:, H * 128:H * 128 + H],
                                        scalar1=1e-9, scalar2=None,
                                        op0=mybir.AluOpType.max)
                dr = spool.tile([128, H], f32, tag="dr")
                nc.vector.reciprocal(dr[:], dmx[:])
                gtsb = epool.tile([128, H * 128], bf16, tag="gtsb")
                nc.scalar.copy(gtsb[:], gt[:, 0:H * 128])
                pp = pspp.tile([128, H * F], f32, tag="pp")
                for c in range(H):
                    nc.tensor.matmul(out=pp[:, c * F:(c + 1) * F],
                                     lhsT=gtsb[:, c * 128:(c + 1) * 128],
                                     rhs=wfold[:, c * F:(c + 1) * F],
                                     start=(c == 0), stop=(c == H - 1))
                xa = fpool.tile([128, F], f32, tag="xa")
                nc.scalar.activation(xa[:], pp[:, 0:F],
                                     mybir.ActivationFunctionType.Copy,
                                     scale=dr[:, 0:1])
                xb = fpool.tile([128, F], f32, tag="xb")
                nc.scalar.activation(xb[:], pp[:, F:2 * F],
                                     mybir.ActivationFunctionType.Copy,
                                     scale=dr[:, 1:2])
                xc_ = fpool.tile([128, F], f32, tag="xc_")
                nc.vector.tensor_scalar_mul(xc_[:], pp[:, 2 * F:3 * F], dr[:, 2:3])
                s1 = fpool.tile([128, F], f32, tag="s1")
                nc.gpsimd.tensor_tensor(out=s1[:], in0=xa[:], in1=xb[:], op=mybir.AluOpType.add)
                s2 = fpool.tile([128, F], f32, tag="s2")
                nc.gpsimd.tensor_tensor(out=s2[:], in0=s1[:], in1=xc_[:], op=mybir.AluOpType.add)
                s3 = fpool.tile([128, F], f32, tag="s3")
                nc.gpsimd.tensor_tensor(out=s3[:], in0=s2[:], in1=hw[:], op=mybir.AluOpType.add)
                x = fpool.tile([128, F], f32, tag="x")
                nc.gpsimd.tensor_tensor(out=x[:], in0=s3[:], in1=bconst[:], op=mybir.AluOpType.add)

                # LayerNorm + relu
                jnk = fpool.tile([128, F], f32, tag="jnk")
                sm = fpool.tile([128, 1], f32, tag="sm")
                nc.scalar.activation(jnk[:], x[:], mybir.ActivationFunctionType.Identity,
                                     accum_out=sm[:, 0:1])
                nmu = fpool.tile([128, 1], f32, tag="nmu")
                nc.vector.tensor_scalar_mul(nmu[:], sm[:], -1.0 / F)
                xm = fpool.tile([128, F], f32, tag="xm")
                nc.scalar.activation(xm[:], x[:], mybir.ActivationFunctionType.Identity,
                                     bias=nmu[:, 0:1], scale=1.0)
                sq = fpool.tile([128, F], f32, tag="sq")
                vs = fpool.tile([128, 1], f32, tag="vs")
                nc.scalar.activation(sq[:], xm[:], mybir.ActivationFunctionType.Square,
                                     accum_out=vs[:, 0:1])
                vp = fpool.tile([128, 1], f32, tag="vp")
                nc.vector.tensor_scalar(out=vp[:], in0=vs[:], scalar1=1.0 / F,
                                        scalar2=LN_EPS, op0=mybir.AluOpType.mult,
                                        op1=mybir.AluOpType.add)
                lvp = fpool.tile([128, 1], f32, tag="lvp")
                nc.scalar.activation(lvp[:], vp[:], mybir.ActivationFunctionType.Ln)
                si = fpool.tile([128, 1], f32, tag="si")
                nc.scalar.activation(si[:], lvp[:], mybir.ActivationFunctionType.Exp,
                                     scale=-0.5)
                y1 = fpool.tile([128, F], f32, tag="y1")
                nc.scalar.activation(y1[:], xm[:], mybir.ActivationFunctionType.Copy,
                                     scale=si[:, 0:1])
                y2 = fpool.tile([128, F], f32, tag="y2")
                nc.gpsimd.tensor_tensor(out=y2[:], in0=y1[:], in1=gam[:], op=mybir.AluOpType.mult)
                y3 = fpool.tile([128, F], f32, tag="y3")
                nc.gpsimd.tensor_tensor(out=y3[:], in0=y2[:], in1=bet[:], op=mybir.AluOpType.add)
                y4 = fpool.tile([128, F], f32, tag="y4")
                nc.scalar.activation(y4[:], y3[:], mybir.ActivationFunctionType.Relu)
                nc.sync.dma_start(outy[w * 128:(w + 1) * 128, :], y4[:])

    nc.compile()
    return nc


def _host_prep(h, src, dst, W_node, b_node, att, w_scale, bias, ln_gamma, ln_beta):
    src = np.asarray(src).astype(np.int64)
    dst = np.asarray(dst).astype(np.int64)
    h = np.asarray(h, dtype=np.float32)
    W_node = np.asarray(W_node, dtype=np.float32)
    b_node = np.asarray(b_node, dtype=np.float32)
    att = np.asarray(att, dtype=np.float32)
    w_scale = np.asarray(w_scale, dtype=np.float32)
    bias = np.asarray(bias, dtype=np.float32)
    ln_gamma = np.asarray(ln_gamma, dtype=np.float32)
    ln_beta = np.asarray(ln_beta, dtype=np.float32)

    deg = np.bincount(dst, minlength=N_NODES)

    # per-core window assignment (balance edge load across NWIN windows)
    win_of = np.zeros(N_NODES, np.int32)
    slot_of = np.zeros(N_NODES, np.int32)
    nodeid = np.zeros((NCORES, NWIN, 128), np.int64)
    valid = np.zeros((NCORES, NWIN, 128), bool)
    maxload = 0
    for k in range(NCORES):
        nodes = np.arange(k * NPC, (k + 1) * NPC)
        order = nodes[np.argsort(-deg[nodes], kind="stable")]
        load = np.zeros(NWIN, np.int64)
        cnt = np.zeros(NWIN, np.int64)
        for n in order:
            cand = np.where(cnt < 128)[0]
            b = cand[np.argmin(load[cand])]
            win_of[n] = b
            slot_of[n] = cnt[b]
            nodeid[k, b, cnt[b]] = n
            valid[k, b, cnt[b]] = True
            load[b] += deg[n]
            cnt[b] += 1
        maxload = max(maxload, load.max())
    T = max(1, int(-(-maxload // 128)))

    sidx = np.zeros((NCORES, NWIN, 128, T), np.int64)
    didx = np.zeros((NCORES, NWIN, 128, T), np.int64)
    wrel = np.full((NCORES, NWIN, 128, T), 255.0, np.float32)

    core_of_edge = dst // NPC
    win_of_edge = win_of[dst]
    for k in range(NCORES):
        em = core_of_edge == k
        for w in range(NWIN):
            sel = em & (win_of_edge == w)
            es = src[sel]
            ed = dst[sel]
            ne = es.shape[0]
            cap = T * 128
            assert ne <= cap
            sarr = np.zeros(cap, np.int64)
            darr = np.zeros(cap, np.int64)
            rarr = np.full(cap, 255.0, np.float32)
            sarr[:ne] = es
            darr[:ne] = ed
            rarr[:ne] = slot_of[ed]
            sidx[k, w] = sarr.reshape(T, 128).T
            didx[k, w] = darr.reshape(T, 128).T
            wrel[k, w] = rarr.reshape(T, 128).T

    # weight-derived constants
    Wn3 = W_node.reshape(H, F, F)                 # (c, f_out, g)
    att_l = att[:, :F]
    att_r = att[:, F:]
    Ael = np.einsum('hfg,hf->gh', Wn3, att_l)     # [g, H]
    Aer = np.einsum('hfg,hf->gh', Wn3, att_r)
    welr = np.concatenate([Ael, Aer], axis=1).astype(ml_dtypes.bfloat16)  # [F, 6]
    b3 = b_node.reshape(H, F)
    cel = (b3 * att_l).sum(1)
    cer = (b3 * att_r).sum(1)
    ccr = np.tile((cel + cer)[None, :], (1, T)).astype(ml_dtypes.bfloat16)  # [1, T*H]

    wfold = np.zeros((F, H * F), np.float32)
    for c in range(H):
        Wc = W_node[c * F:(c + 1) * F, :]
        wsc_c = w_scale[c * F:(c + 1) * F, :]
        wfold[:, c * F:(c + 1) * F] = Wc.T @ wsc_c
    bconst_row = b_node @ w_scale + bias

    hbf = h.astype(ml_dtypes.bfloat16)
    iotar3 = np.tile(np.arange(128, dtype=np.float32)[None, :],
                     (128, H)).astype(ml_dtypes.bfloat16)

    common = {
        "welr": np.ascontiguousarray(welr),
        "wfold": np.ascontiguousarray(wfold.astype(ml_dtypes.bfloat16)),
        "iotar3": np.ascontiguousarray(iotar3),
        "ccr": ccr,
        "ones1": np.ones((1, 128), ml_dtypes.bfloat16),
        "gam": np.tile(ln_gamma[None, :], (128, 1)).astype(np.float32),
        "bet": np.tile(ln_beta[None, :], (128, 1)).astype(np.float32),
        "bconst": np.tile(bconst_row[None, :], (128, 1)).astype(np.float32),
    }
    in_maps = []
    for k in range(NCORES):
        he = hbf[sidx[k]]                          # [NWIN, 128, T, F]
        hd = hbf[didx[k]]
        heT = np.ascontiguousarray(he.transpose(0, 3, 2, 1))  # [NWIN, F, T, 128]
        hdT = np.ascontiguousarray(hd.transpose(0, 3, 2, 1))
        hwin = np.zeros((NWIN, 128, F), np.float32)
        hwin[valid[k]] = h[nodeid[k][valid[k]]]
        m = dict(common)
        m["he"] = np.ascontiguousarray(he.reshape(NWIN, 128, T * F))
        m["heT"] = heT.reshape(NWIN, 128, T * F)
        m["hdT"] = hdT.reshape(NWIN, 128, T * F)
        m["hwin"] = hwin
        m["wrel"] = np.ascontiguousarray(wrel[k])
        in_maps.append(m)
    return T, in_maps, nodeid, valid


def kernel(h, src, dst, W_node, b_node, att, w_scale, bias, ln_gamma, ln_beta,
           _want_trace=False):
    T, in_maps, nodeid, valid = _host_prep(
        h, src, dst, W_node, b_node, att, w_scale, bias, ln_gamma, ln_beta)
    if T not in _PROGRAM_CACHE:
        _PROGRAM_CACHE[T] = _build_program(T)
    nc = _PROGRAM_CACHE[T]
    res = run_bass_kernel_spmd(nc, in_maps, list(range(NCORES)), trace=_want_trace)
    out = np.zeros((N_NODES, F), np.float32)
    for k in range(NCORES):
        rows = res.results[k]["outy"].reshape(NWIN, 128, F)
        v = valid[k]
        out[nodeid[k][v]] = rows[v]
    if _want_trace:
        kernel._last_exec_time_ns = res.exec_time_ns
        kernel._last_trace = res.instructions_and_trace
    return out


# revision 30
# speedup vs baseline: 1.0754x; 1.0053x over previous
"""Multi-head GAT layer on 8 Trainium2 NeuronCores (Bass/Tile).

Strategy: nodes sharded 6250/core; each core owns all edges whose dst is in
its shard. Edges are host-binned into 49 windows of 128 dst slots. The host
stages edge-ordered copies of h (the halo exchange) so the device does only
contiguous DMA loads; per window the device computes el/er attention logits
with per-column matmuls accumulating in PSUM, exponentiates, builds weighted
one-hot matrices, and scatter-accumulates sum_e w_e*h[src_e] per dst slot via
matmuls oriented so the result lands feature-major (no transposes). The output
projection uses the folded weight W_c^T @ wsc_c; biases fold into constants.
"""
import sys, os, types, ctypes, contextlib

if '/opt/trn_rl_repo' not in sys.path:
    sys.path.insert(0, '/opt/trn_rl_repo')


def _install_profile_hook():
    try:
        import antenv.axon_hooks  # noqa
        return
    except ImportError:
        pass
    try:
        import antenv
    except ImportError:
        return
    so_path = "/opt/axon/libaxon_pjrt.so"
    hook = None
    if os.path.exists(so_path):
        lib = ctypes.CDLL(so_path)
        if hasattr(lib, "axon_start_nrt_profile"):
            lib.axon_start_nrt_profile.argtypes = [ctypes.POINTER(ctypes.c_int64), ctypes.c_size_t]
            lib.axon_start_nrt_profile.restype = ctypes.c_int64
            lib.axon_stop_nrt_profile.argtypes = [ctypes.c_char_p]
            lib.axon_stop_nrt_profile.restype = ctypes.c_int64

            @contextlib.contextmanager
            def _hook(output_dir, device_ids):
                import jax
                jax.devices()
                if device_ids:
                    ids = (ctypes.c_int64 * len(device_ids))(*device_ids)
                    rc = lib.axon_start_nrt_profile(ids, len(device_ids))
                else:
                    rc = lib.axon_start_nrt_profile(None, 0)
                if rc != 0:
                    raise RuntimeError(f"axon_start_nrt_profile rc={rc}")
                try:
                    yield
                finally:
                    n = lib.axon_stop_nrt_profile(str(output_dir).encode())
                    print(f"ntff profile: {n} file(s) -> {output_dir}", file=sys.stderr)
            hook = _hook
    mod = types.ModuleType("antenv.axon_hooks")
    state = {"hook": hook}
    mod.set_axon_ntff_profile_hook = lambda h: state.__setitem__("hook", h)
    mod.get_axon_ntff_profile_hook = lambda: state["hook"]
    sys.modules["antenv.axon_hooks"] = mod
    antenv.axon_hooks = mod


_install_profile_hook()

import numpy as np
import ml_dtypes
from concourse import bass, bacc, mybir, tile
from concourse.bass_utils import run_bass_kernel_spmd

N_NODES = 50000
F = 128
H = 3
NCORES = 8
NPC = N_NODES // NCORES          # 6250 nodes per core
NWIN = (NPC + 127) // 128        # 49 windows per core
NEG_SLOPE = 0.2
LN_EPS = 1e-5

f32 = mybir.dt.float32
bf16 = mybir.dt.bfloat16

_PROGRAM_CACHE = {}


def _build_program(T):
    nc = bacc.Bacc("TRN2", target_bir_lowering=False, debug=False,
                   enable_asserts=False, num_devices=NCORES,
                   dynamic_dma_scratch_size=16384)

    he_in = nc.dram_tensor("he", [NWIN, 128, T * F], bf16, kind="ExternalInput").ap()
    heT_in = nc.dram_tensor("heT", [NWIN, 128, T * F], bf16, kind="ExternalInput").ap()
    hdT_in = nc.dram_tensor("hdT", [NWIN, 128, T * F], bf16, kind="ExternalInput").ap()
    hwin_in = nc.dram_tensor("hwin", [NWIN, 128, F], f32, kind="ExternalInput").ap()
    wrel_in = nc.dram_tensor("wrel", [NWIN, 128, T], f32, kind="ExternalInput").ap()
    welr_in = nc.dram_tensor("welr", [F, 6], bf16, kind="ExternalInput").ap()
    wfold_in = nc.dram_tensor("wfold", [F, H * F], bf16, kind="ExternalInput").ap()
    iotar3_in = nc.dram_tensor("iotar3", [128, H * 128], bf16, kind="ExternalInput").ap()
    ccr_in = nc.dram_tensor("ccr", [1, T * H], bf16, kind="ExternalInput").ap()
    ones1_in = nc.dram_tensor("ones1", [1, 128], bf16, kind="ExternalInput").ap()
    gam_in = nc.dram_tensor("gam", [128, F], f32, kind="ExternalInput").ap()
    bet_in = nc.dram_tensor("bet", [128, F], f32, kind="ExternalInput").ap()
    bconst_in = nc.dram_tensor("bconst", [128, F], f32, kind="ExternalInput").ap()

    outy = nc.dram_tensor("outy", [NWIN * 128, F], f32, kind="ExternalOutput").ap()

    with tile.TileContext(nc) as tc:
        with (
            tc.tile_pool(name="const", bufs=1) as cpool,
            tc.tile_pool(name="gath", bufs=4) as gpool,
            tc.tile_pool(name="edge", bufs=8) as epool,
            tc.tile_pool(name="small", bufs=5) as spool,
            tc.tile_pool(name="fin", bufs=5) as fpool,
            tc.tile_pool(name="psat", bufs=2, space="PSUM") as psat,
            tc.tile_pool(name="psgt", bufs=3, space="PSUM") as psgt,
            tc.tile_pool(name="pspp", bufs=3, space="PSUM") as pspp,
        ):
            # ---- constants
            welr = cpool.tile([F, 6], bf16)
            nc.sync.dma_start(welr[:], welr_in[:])
            wfold = cpool.tile([F, H * F], bf16)
            nc.sync.dma_start(wfold[:], wfold_in[:])
            iotar3 = cpool.tile([128, H * 128], bf16)
            nc.sync.dma_start(iotar3[:], iotar3_in[:])
            ccr = cpool.tile([1, T * H], bf16)
            nc.sync.dma_start(ccr[:], ccr_in[:])
            ones1 = cpool.tile([1, 128], bf16)
            nc.sync.dma_start(ones1[:], ones1_in[:])
            gam = cpool.tile([128, F], f32)
            nc.sync.dma_start(gam[:], gam_in[:])
            bet = cpool.tile([128, F], f32)
            nc.sync.dma_start(bet[:], bet_in[:])
            bconst = cpool.tile([128, F], f32)
            nc.sync.dma_start(bconst[:], bconst_in[:])
            onescol = cpool.tile([128, 1], bf16)
            nc.vector.memset(onescol[:], 1.0)
            pt02 = cpool.tile([128, 64], f32)
            nc.vector.memset(pt02[:], NEG_SLOPE)

            def stage_attn(w):
                ghe = gpool.tile([128, T, F], bf16, tag="ghe")
                nc.sync.dma_start(ghe[:].rearrange("p t f -> p (t f)"), he_in[w])
                gheT = gpool.tile([128, T, F], bf16, tag="gheT")
                nc.scalar.dma_start(gheT[:].rearrange("p t f -> p (t f)"), heT_in[w])
                gdT = gpool.tile([128, T, F], bf16, tag="gdT")
                nc.sync.dma_start(gdT[:].rearrange("p t f -> p (t f)"), hdT_in[w])
                hw = spool.tile([128, F], f32, tag="hw")
                nc.scalar.dma_start(hw[:], hwin_in[w])
                wr = spool.tile([128, T], f32, tag="wr")
                nc.sync.dma_start(wr[:], wrel_in[w])

                # attention logits el[src]+er[dst]+bias accumulate in PSUM
                atp = psat.tile([128, T * H], f32, tag="at")
                nc.tensor.matmul(out=atp[:], lhsT=ones1[:], rhs=ccr[:],
                                 start=True, stop=False)
                for j in range(T):
                    nc.tensor.matmul(out=atp[:, j * H:(j + 1) * H],
                                     lhsT=gheT[:, j, :], rhs=welr[:, 0:3],
                                     start=False, stop=False)
                    nc.tensor.matmul(out=atp[:, j * H:(j + 1) * H],
                                     lhsT=gdT[:, j, :], rhs=welr[:, 3:6],
                                     start=False, stop=(j == T - 1))

                # ew = exp(leaky(attn))
                atsb = spool.tile([128, T * H], f32, tag="atsb")
                nc.scalar.copy(atsb[:], atp[:])
                at2 = spool.tile([128, T * H], f32, tag="at2")
                nc.gpsimd.tensor_tensor(out=at2[:], in0=atsb[:], in1=pt02[:, 0:T * H],
                                        op=mybir.AluOpType.mult)
                at3 = spool.tile([128, T * H], f32, tag="at3")
                nc.vector.tensor_tensor(out=at3[:], in0=atsb[:], in1=at2[:],
                                        op=mybir.AluOpType.max)
                ew = spool.tile([128, T, H], f32, tag="ew")
                nc.scalar.activation(ew[:].rearrange("p t c -> p (t c)"), at3[:],
                                     mybir.ActivationFunctionType.Exp)
                return ghe, hw, wr, ew

            cur = stage_attn(0)
            for w in range(NWIN):
                ghe, hw, wr, ew = cur
                if w + 1 < NWIN:
                    cur = stage_attn(w + 1)

                # scatter: GT[f, c*128+slot] += sum_e ghe[e,f] * OW_c[e,slot]
                # den_c[slot] accumulates in cols 384:387 of the same PSUM bank
                gt = psgt.tile([128, H * 128 + H], f32, tag="gt")
                for j in range(T):
                    ow = epool.tile([128, H * 128], bf16, tag="ow")
                    for c in range(H):
                        nc.vector.tensor_scalar(
                            out=ow[:, c * 128:(c + 1) * 128],
                            in0=iotar3[:, c * 128:(c + 1) * 128],
                            scalar1=wr[:, j:j + 1], scalar2=ew[:, j, c:c + 1],
                            op0=mybir.AluOpType.is_equal, op1=mybir.AluOpType.mult)
                    nc.tensor.matmul(out=gt[:, 0:H * 128], lhsT=ghe[:, j, :], rhs=ow[:],
                                     start=(j == 0), stop=False)
                    for c in range(H):
                        nc.tensor.matmul(out=gt[:, H * 128 + c:H * 128 + c + 1],
                                         lhsT=ow[:, c * 128:(c + 1) * 128],
                                         rhs=onescol[:], start=False,
                                         stop=(j == T - 1 and c == H - 1))

                # epilogue: x = sum_c (GT_c^T @ wfold_c) / den_c + hwin + bconst
                dmx = spool.tile([128, H], f32, tag="dmx")
                nc.vector.tensor_scalar(out=dmx[:], in0=gt[:, H * 128:H * 128 + H],
                                        scalar1=1e-9, scalar2=None,
                                        op0=mybir.AluOpType.max)
                dr = spool.tile([128, H], f32, tag="dr")
                nc.vector.reciprocal(dr[:], dmx[:])
                gtsb = epool.tile([128, H * 128], bf16, tag="gtsb")
                nc.scalar.copy(gtsb[:], gt[:, 0:H * 128])
                pp = pspp.tile([128, H * F], f32, tag="pp")
                for c in range(H):
                    nc.tensor.matmul(out=pp[:, c * F:(c + 1) * F],
                                     lhsT=gtsb[:, c * 128:(c + 1) * 128],
                                     rhs=wfold[:, c * F:(c + 1) * F],
                                     start=(c == 0), stop=(c == H - 1))
                xa = fpool.tile([128, F], f32, tag="xa")
                nc.scalar.activation(xa[:], pp[:, 0:F],
                                     mybir.ActivationFunctionType.Copy,
                                     scale=dr[:, 0:1])
                xb = fpool.tile([128, F], f32, tag="xb")
                nc.scalar.activation(xb[:], pp[:, F:2 * F],
                                     mybir.ActivationFunctionType.Copy,
                                     scale=dr[:, 1:2])
                xc_ = fpool.tile([128, F], f32, tag="xc_")
                nc.vector.tensor_scalar_mul(xc_[:], pp[:, 2 * F:3 * F], dr[:, 2:3])
                s1 = fpool.tile([128, F], f32, tag="s1")
                nc.gpsimd.tensor_tensor(out=s1[:], in0=xa[:], in1=xb[:], op=mybir.AluOpType.add)
                s2 = fpool.tile([128, F], f32, tag="s2")
                nc.gpsimd.tensor_tensor(out=s2[:], in0=s1[:], in1=xc_[:], op=mybir.AluOpType.add)
                s3 = fpool.tile([128, F], f32, tag="s3")
                nc.gpsimd.tensor_tensor(out=s3[:], in0=s2[:], in1=hw[:], op=mybir.AluOpType.add)
                x = fpool.tile([128, F], f32, tag="x")
                nc.gpsimd.tensor_tensor(out=x[:], in0=s3[:], in1=bconst[:], op=mybir.AluOpType.add)

                # LayerNorm + relu
                jnk = fpool.tile([128, F], f32, tag="jnk")
                sm = fpool.tile([128, 1], f32, tag="sm")
                nc.scalar.activation(jnk[:], x[:], mybir.ActivationFunctionType.Identity,
                                     accum_out=sm[:, 0:1])
                nmu = fpool.tile([128, 1], f32, tag="nmu")
                nc.vector.tensor_scalar_mul(nmu[:], sm[:], -1.0 / F)
                xm = fpool.tile([128, F], f32, tag="xm")
                nc.scalar.activation(xm[:], x[:], mybir.ActivationFunctionType.Identity,
                                     bias=nmu[:, 0:1], scale=1.0)
                sq = fpool.tile([128, F], f32, tag="sq")
                vs = fpool.tile([128, 1], f32, tag="vs")
                nc.scalar.activation(sq[:], xm[:], mybir.ActivationFunctionType.Square,
                                     accum_out=vs[:, 0:1])
                vp = fpool.tile([128, 1], f32, tag="vp")
                nc.vector.tensor_scalar(out=vp[:], in0=vs[:], scalar1=1.0 / F,
                                        scalar2=LN_EPS, op0=mybir.AluOpType.mult,
                                        op1=mybir.AluOpType.add)
                lvp = fpool.tile([128, 1], f32, tag="lvp")
                nc.scalar.activation(lvp[:], vp[:], mybir.ActivationFunctionType.Ln)
                si = fpool.tile([128, 1], f32, tag="si")
                nc.scalar.activation(si[:], lvp[:], mybir.ActivationFunctionType.Exp,
                                     scale=-0.5)
                y1 = fpool.tile([128, F], f32, tag="y1")
                nc.scalar.activation(y1[:], xm[:], mybir.ActivationFunctionType.Copy,
                                     scale=si[:, 0:1])
                y2 = fpool.tile([128, F], f32, tag="y2")
                nc.gpsimd.tensor_tensor(out=y2[:], in0=y1[:], in1=gam[:], op=mybir.AluOpType.mult)
                y3 = fpool.tile([128, F], f32, tag="y3")
                nc.gpsimd.tensor_tensor(out=y3[:], in0=y2[:], in1=bet[:], op=mybir.AluOpType.add)
                y4 = fpool.tile([128, F], f32, tag="y4")
                nc.scalar.activation(y4[:], y3[:], mybir.ActivationFunctionType.Relu)
                nc.sync.dma_start(outy[w * 128:(w + 1) * 128, :], y4[:])

    nc.compile()
    return nc


def _host_prep(h, src, dst, W_node, b_node, att, w_scale, bias, ln_gamma, ln_beta):
    src = np.asarray(src).astype(np.int64)
    dst = np.asarray(dst).astype(np.int64)
    h = np.asarray(h, dtype=np.float32)
    W_node = np.asarray(W_node, dtype=np.float32)
    b_node = np.asarray(b_node, dtype=np.float32)
    att = np.asarray(att, dtype=np.float32)
    w_scale = np.asarray(w_scale, dtype=np.float32)
    bias = np.asarray(bias, dtype=np.float32)
    ln_gamma = np.asarray(ln_gamma, dtype=np.float32)
    ln_beta = np.asarray(ln_beta, dtype=np.float32)

    deg = np.bincount(dst, minlength=N_NODES)

    # per-core window assignment (balance edge load across NWIN windows)
    win_of = np.zeros(N_NODES, np.int32)
    slot_of = np.zeros(N_NODES, np.int32)
    nodeid = np.zeros((NCORES, NWIN, 128), np.int64)
    valid = np.zeros((NCORES, NWIN, 128), bool)
    maxload = 0
    for k in range(NCORES):
        nodes = np.arange(k * NPC, (k + 1) * NPC)
        order = nodes[np.argsort(-deg[nodes], kind="stable")]
        load = np.zeros(NWIN, np.int64)
        cnt = np.zeros(NWIN, np.int64)
        for n in order:
            cand = np.where(cnt < 128)[0]
            b = cand[np.argmin(load[cand])]
            win_of[n] = b
            slot_of[n] = cnt[b]
            nodeid[k, b, cnt[b]] = n
            valid[k, b, cnt[b]] = True
            load[b] += deg[n]
            cnt[b] += 1
        maxload = max(maxload, load.max())
    T = max(1, int(-(-maxload // 128)))

    sidx = np.zeros((NCORES, NWIN, 128, T), np.int64)
    didx = np.zeros((NCORES, NWIN, 128, T), np.int64)
    wrel = np.full((NCORES, NWIN, 128, T), 255.0, np.float32)

    core_of_edge = dst // NPC
    win_of_edge = win_of[dst]
    for k in range(NCORES):
        em = core_of_edge == k
        for w in range(NWIN):
            sel = em & (win_of_edge == w)
            es = src[sel]
            ed = dst[sel]
            ne = es.shape[0]
            cap = T * 128
            assert ne <= cap
            sarr = np.zeros(cap, np.int64)
            darr = np.zeros(cap, np.int64)
            rarr = np.full(cap, 255.0, np.float32)
            sarr[:ne] = es
            darr[:ne] = ed
            rarr[:ne] = slot_of[ed]
            sidx[k, w] = sarr.reshape(T, 128).T
            didx[k, w] = darr.reshape(T, 128).T
            wrel[k, w] = rarr.reshape(T, 128).T

    # weight-derived constants
    Wn3 = W_node.reshape(H, F, F)                 # (c, f_out, g)
    att_l = att[:, :F]
    att_r = att[:, F:]
    Ael = np.einsum('hfg,hf->gh', Wn3, att_l)     # [g, H]
    Aer = np.einsum('hfg,hf->gh', Wn3, att_r)
    welr = np.concatenate([Ael, Aer], axis=1).astype(ml_dtypes.bfloat16)  # [F, 6]
    b3 = b_node.reshape(H, F)
    cel = (b3 * att_l).sum(1)
    cer = (b3 * att_r).sum(1)
    ccr = np.tile((cel + cer)[None, :], (1, T)).astype(ml_dtypes.bfloat16)  # [1, T*H]

    wfold = np.zeros((F, H * F), np.float32)
    for c in range(H):
        Wc = W_node[c * F:(c + 1) * F, :]
        wsc_c = w_scale[c * F:(c + 1) * F, :]
        wfold[:, c * F:(c + 1) * F] = Wc.T @ wsc_c
    bconst_row = b_node @ w_scale + bias

    hbf = h.astype(ml_dtypes.bfloat16)
    iotar3 = np.tile(np.arange(128, dtype=np.float32)[None, :],
                     (128, H)).astype(ml_dtypes.bfloat16)

    common = {
        "welr": np.ascontiguousarray(welr),
        "wfold": np.ascontiguousarray(wfold.astype(ml_dtypes.bfloat16)),
        "iotar3": np.ascontiguousarray(iotar3),
        "ccr": ccr,
        "ones1": np.ones((1, 128), ml_dtypes.bfloat16),
        "gam": np.tile(ln_gamma[None, :], (128, 1)).astype(np.float32),
        "bet": np.tile(ln_beta[None, :], (128, 1)).astype(np.float32),
        "bconst": np.tile(bconst_row[None, :], (128, 1)).astype(np.float32),
    }
    in_maps = []
    for k in range(NCORES):
        he = hbf[sidx[k]]                          # [NWIN, 128, T, F]
        hd = hbf[didx[k]]
        heT = np.ascontiguousarray(he.transpose(0, 3, 2, 1))  # [NWIN, F, T, 128]
        hdT = np.ascontiguousarray(hd.transpose(0, 3, 2, 1))
        hwin = np.zeros((NWIN, 128, F), np.float32)
        hwin[valid[k]] = h[nodeid[k][valid[k]]]
        m = dict(common)
        m["he"] = np.ascontiguousarray(he.reshape(NWIN, 128, T * F))
        m["heT"] = heT.reshape(NWIN, 128, T * F)
        m["hdT"] = hdT.reshape(NWIN, 128, T * F)
        m["hwin"] = hwin
        m["wrel"] = np.ascontiguousarray(wrel[k])
        in_maps.append(m)
    return T, in_maps, nodeid, valid


def kernel(h, src, dst, W_node, b_node, att, w_scale, bias, ln_gamma, ln_beta,
           _want_trace=False):
    T, in_maps, nodeid, valid = _host_prep(
        h, src, dst, W_node, b_node, att, w_scale, bias, ln_gamma, ln_beta)
    if T not in _PROGRAM_CACHE:
        _PROGRAM_CACHE[T] = _build_program(T)
    nc = _PROGRAM_CACHE[T]
    res = run_bass_kernel_spmd(nc, in_maps, list(range(NCORES)), trace=_want_trace)
    out = np.zeros((N_NODES, F), np.float32)
    for k in range(NCORES):
        rows = res.results[k]["outy"].reshape(NWIN, 128, F)
        v = valid[k]
        out[nodeid[k][v]] = rows[v]
    if _want_trace:
        kernel._last_exec_time_ns = res.exec_time_ns
        kernel._last_trace = res.instructions_and_trace
    return out
